# revision 23
# baseline (speedup 1.0000x reference)
"""Trainium2 Bass kernel for DiagLinearRNNCell.

Reference computation (replicated exactly, including the 1e-12 clamp):
    a = tanh(raw_a)                         # [H]
    z = x @ W.T + b                         # [B,T,H]
    p[t] = a^(t+1)  (f32 cumprod)           # [T,H]
    v = cumsum_t(z / max(p, 1e-12))         # [B,T,H]
    h = v * p + p * h0                      # [B,T,H]

Because a ~ 0.95, p underflows the 1e-12 clamp around t ~ 540, so the
reference is NOT the plain linear recurrence for large t.  It is, however,
exactly equivalent (in exact arithmetic) to the *stable* recurrence

    h[t] = a * h[t-1] + d[t] * z[t],   h[-1] = h0,
    d[t] = 1            where p[t] >= 1e-12
         = p[t] * 1e12  where p[t] <  1e-12

which is what the device computes:  z via TensorE matmuls (W stationary,
x moving, channels on partitions, time on the free axis), u = d*z via a
VectorE tensor_tensor multiply, and the recurrence itself via the VectorE
tensor_tensor_scan instruction (state = a*state + u along the free axis).

Sharding: data-parallel over batch, 2 sequences per core on 8 cores.
The d table is precomputed on the host (it only depends on raw_a) and the
output is produced channel-major ([b, hc, hh, t]) then transposed back to
[B, T, H] on the host during the unshard step.
"""

import os
from contextlib import ExitStack

import numpy as np

import concourse.bass as bass
import concourse.bass_utils as _bu
import concourse.tile as tile
from concourse import bacc, mybir
from concourse.bass_utils import run_bass_kernel_spmd

B, T, D, H = 16, 1024, 512, 1024
NCORES = 8
BLOC = B // NCORES          # sequences per core
DC, HC = D // 128, H // 128  # 128-chunk counts

# moving-operand dtype: float32 (exact, 4 cyc/row) or float32r (~2^-13, 2 cyc/row)
MM_F32 = os.environ.get("KERNEL_MM_F32") == "1"
MM_DTYPE = mybir.dt.float32 if MM_F32 else mybir.dt.float32r
# how many of the 16 (b,hc) elementwise multiplies go to GpSimd instead of DVE
GP_MULT = int(os.environ.get("KERNEL_GP_MULT", "0"))

if os.environ.get("KERNEL_LDW_OPT", "1") == "1" and not getattr(_bu, "_ldw_patched", False):
    _orig_run_command = _bu.run_command

    def _patched_run_command(argv, **kw):
        argv = ["--enable-ldw-opt=true" if a == "--enable-ldw-opt=false" else a
                for a in argv]
        return _orig_run_command(argv, **kw)

    _bu.run_command = _patched_run_command
    _bu._ldw_patched = True

_cache: dict = {}


def _build(clean, has_bias):
    """Build + compile the SPMD program. clean[hc]: d[0:512, hc-chunk] == 1."""
    nc = bacc.Bacc("TRN2", target_bir_lowering=False, debug=False)

    xT = nc.dram_tensor("xT", [DC, 128, BLOC * T], MM_DTYPE, kind="ExternalInput")
    WT = nc.dram_tensor("WT", [DC, 128, H], MM_DTYPE, kind="ExternalInput")
    dT = nc.dram_tensor("dT", [HC, 128, T], mybir.dt.float32, kind="ExternalInput")
    aT = nc.dram_tensor("aT", [128, HC], mybir.dt.float32, kind="ExternalInput")
    h0T = nc.dram_tensor("h0T", [128, HC * BLOC], mybir.dt.float32, kind="ExternalInput")
    if has_bias:
        bT = nc.dram_tensor("bT", [128, HC], mybir.dt.float32, kind="ExternalInput")
    hT = nc.dram_tensor("hT", [BLOC, HC, 128, T], mybir.dt.float32, kind="ExternalOutput")

    with tile.TileContext(nc) as tc, ExitStack() as ctx:
        const = ctx.enter_context(tc.tile_pool(name="const", bufs=1))
        dpool = ctx.enter_context(tc.tile_pool(name="dpool", bufs=HC))
        upool = ctx.enter_context(tc.tile_pool(name="upool", bufs=4))
        hpool = ctx.enter_context(tc.tile_pool(name="hpool", bufs=4))
        psum = ctx.enter_context(tc.tile_pool(name="psum", bufs=4, space="PSUM"))

        # per-d-chunk tiles; fine-grained first-chunk DMAs (spread across two
        # issue engines) so the first matmuls can start as early as possible
        x_sb = [const.tile([128, BLOC * T], MM_DTYPE, name=f"x{dc}", tag=f"x{dc}")
                for dc in range(DC)]
        w_sb = [const.tile([128, H], MM_DTYPE, name=f"w{dc}", tag=f"w{dc}")
                for dc in range(DC)]
        # order input pieces by first use, spread across engines/queues
        # (one HW DMA queue sustains only ~21 GB/s, so parallelism is key)
        engs = [nc.sync, nc.scalar, nc.gpsimd]
        ei = 0

        def _dma_in(dst, src):
            nonlocal ei
            engs[ei % len(engs)].dma_start(dst, src)
            ei += 1

        # tiny tables first (single coalesced DMAs): the first scan needs them
        a_sb = const.tile([128, HC], mybir.dt.float32)
        nc.scalar.dma_start(a_sb[:], aT.ap())
        h0_sb = const.tile([128, HC * BLOC], mybir.dt.float32)
        nc.sync.dma_start(h0_sb[:], h0T.ap())
        if has_bias:
            bias_sb = const.tile([128, HC], mybir.dt.float32)
            nc.scalar.dma_start(bias_sb[:], bT.ap())
        # d tiles for the first two hc iterations, split across queues, ahead
        # of the x/w bulk: the first scans/mults block on these
        d_tiles = {}
        for hc in range(HC):
            d_tiles[hc] = dpool.tile([128, T], mybir.dt.float32, name=f"d{hc}",
                                     tag="d")

        def _load_d(hc):
            t_lo = 512 if (clean[hc] and not has_bias) else 0
            for c0 in range(t_lo, T, 256):
                _dma_in(d_tiles[hc][:, c0:c0 + 256], dT.ap()[hc, :, c0:c0 + 256])

        _load_d(0)
        _load_d(1)
        for dc in range(DC):  # first weight slice + first moving slice per dc
            _dma_in(w_sb[dc][:, 0:256], WT.ap()[dc, :, 0:256])
            _dma_in(x_sb[dc][:, 0:256], xT.ap()[dc, :, 0:256])
            _dma_in(x_sb[dc][:, 256:512], xT.ap()[dc, :, 256:512])
        for dc in range(DC):
            _dma_in(w_sb[dc][:, 256:H], WT.ap()[dc, :, 256:H])
            for q in range(1, 4):
                _dma_in(x_sb[dc][:, q * 512:(q + 1) * 512],
                        xT.ap()[dc, :, q * 512:(q + 1) * 512])
        for hc in range(2, HC):
            _load_d(hc)

        mult_idx = 0
        for hc in range(HC):
            d_sb = d_tiles[hc]

            # weight-reuse order: one weight tile per (hc, dc) serves BLOC*2 MMs
            zp = [psum.tile([128, T], mybir.dt.float32, name=f"zp{hc}_{b2}", tag="z")
                  for b2 in range(BLOC)]
            for dc in range(DC):
                w_sl = w_sb[dc][:, hc * 128:(hc + 1) * 128]
                for b in range(BLOC):
                    for tt in range(T // 512):
                        nc.tensor.matmul(
                            zp[b][:, tt * 512:(tt + 1) * 512],
                            w_sl,
                            x_sb[dc][:, b * T + tt * 512: b * T + (tt + 1) * 512],
                            start=(dc == 0), stop=(dc == DC - 1),
                        )

            for b in range(BLOC):
                h_t = hpool.tile([128, T], mybir.dt.float32, tag="h")
                a_bc = a_sb[:, hc:hc + 1].to_broadcast([128, T])
                h0_col = h0_sb[:, hc * BLOC + b: hc * BLOC + b + 1]
                mult_idx += 1

                if has_bias:
                    u_t = upool.tile([128, T], mybir.dt.float32, tag="u")
                    nc.vector.scalar_tensor_tensor(
                        out=u_t[:], in0=zp[b][:], scalar=bias_sb[:, hc:hc + 1],
                        in1=d_sb[:], op0=mybir.AluOpType.add,
                        op1=mybir.AluOpType.mult,
                    )
                    nc.vector.tensor_tensor_scan(
                        out=h_t[:], data0=a_bc, data1=u_t[:], initial=h0_col,
                        op0=mybir.AluOpType.mult, op1=mybir.AluOpType.add,
                    )
                    nc.gpsimd.dma_start(hT.ap()[b, hc], h_t[:])
                elif clean[hc]:
                    # first half: d == 1, scan straight out of PSUM, and ship
                    # the finished half while the second half still cooks
                    nc.vector.tensor_tensor_scan(
                        out=h_t[:, 0:512],
                        data0=a_bc[:, 0:512], data1=zp[b][:, 0:512],
                        initial=h0_col,
                        op0=mybir.AluOpType.mult, op1=mybir.AluOpType.add,
                    )
                    nc.gpsimd.dma_start(hT.ap()[b, hc, :, 0:512], h_t[:, 0:512])
                    u_t = upool.tile([128, 512], mybir.dt.float32, tag="u")
                    nc.vector.tensor_mul(u_t[:], zp[b][:, 512:T], d_sb[:, 512:T])
                    nc.vector.tensor_tensor_scan(
                        out=h_t[:, 512:T],
                        data0=a_bc[:, 512:T], data1=u_t[:],
                        initial=h_t[:, 511:512],
                        op0=mybir.AluOpType.mult, op1=mybir.AluOpType.add,
                    )
                    nc.gpsimd.dma_start(hT.ap()[b, hc, :, 512:T], h_t[:, 512:T])
                else:
                    u_t = upool.tile([128, T], mybir.dt.float32, tag="uf")
                    nc.vector.tensor_mul(u_t[:], zp[b][:], d_sb[:])
                    nc.vector.tensor_tensor_scan(
                        out=h_t[:], data0=a_bc, data1=u_t[:], initial=h0_col,
                        op0=mybir.AluOpType.mult, op1=mybir.AluOpType.add,
                    )
                    nc.gpsimd.dma_start(hT.ap()[b, hc], h_t[:])

    nc.compile()
    return nc


def _host_prep(x, h0, raw_a, W, b):
    a = np.tanh(raw_a.astype(np.float32))                       # [H] f32
    A = np.broadcast_to(a, (T, H))
    p = np.cumprod(A, axis=0, dtype=np.float32)                 # [T,H] = a^(t+1)
    d = np.where(p < np.float32(1e-12), p * np.float32(1e12),
                 np.float32(1.0)).astype(np.float32)            # [T,H]
    clean = tuple(bool(np.all(d[0:512, hc * 128:(hc + 1) * 128] == 1.0))
                  for hc in range(HC))
    has_bias = bool(np.any(b))

    shared = {
        "WT": np.ascontiguousarray(W.T.reshape(DC, 128, H), dtype=np.float32),
        "dT": np.ascontiguousarray(d.T.reshape(HC, 128, T)),
        # a_sb[hh, hc] = a[hc*128+hh]
        "aT": np.ascontiguousarray(a.reshape(HC, 128).T),
    }
    if has_bias:
        shared["bT"] = np.ascontiguousarray(
            b.astype(np.float32).reshape(HC, 128).T)

    in_maps = []
    for i in range(NCORES):
        xc = x[i * BLOC:(i + 1) * BLOC]                          # [BLOC,T,D]
        xT_np = np.ascontiguousarray(
            xc.transpose(2, 0, 1).reshape(DC, 128, BLOC * T), dtype=np.float32)
        h0c = h0[i * BLOC:(i + 1) * BLOC]                        # [BLOC,H]
        # h0_sb[hh, hc*BLOC+b] = h0[b, hc*128+hh]
        h0T_np = np.ascontiguousarray(
            h0c.T.reshape(HC, 128, BLOC).transpose(1, 0, 2).reshape(128, HC * BLOC),
            dtype=np.float32)
        in_maps.append({"xT": xT_np, "h0T": h0T_np, **shared})
    return in_maps, clean, has_bias


def kernel(x, h0, raw_a, W, b, _trace=False):
    in_maps, clean, has_bias = _host_prep(
        np.asarray(x), np.asarray(h0), np.asarray(raw_a), np.asarray(W),
        np.asarray(b))

    key = (str(MM_DTYPE), GP_MULT, clean, has_bias)
    if key not in _cache:
        _cache[key] = _build(clean, has_bias)
    nc = _cache[key]

    res = run_bass_kernel_spmd(nc, in_maps, list(range(NCORES)), trace=_trace)

    out = np.empty((B, T, H), np.float32)
    for i in range(NCORES):
        arr = res.results[i]["hT"]                    # [BLOC, HC, 128, T]
        out[i * BLOC:(i + 1) * BLOC] = (
            arr.transpose(0, 3, 1, 2).reshape(BLOC, T, H))
    if _trace:
        return out, res
    return out
